# revision 60
# baseline (speedup 1.0000x reference)
"""Coarse-Fine self-attention layer on 8 Trainium2 NeuronCores.

Data-parallel over batch: 16 batches -> 2 per core. Weights replicated.

Transposed-attention formulation (keys on partitions everywhere):
  - energy is computed TRANSPOSED: E^T[m,n] = x_k^T x_q via lhsT=xks chunks,
    so the per-key positional term kp[m] and the global K shift ride the
    activation-exp bias (per-partition) for free, and no SBUF<->SBUF DMA
    transposes of att/x_v are needed at all.
  - x_v is produced directly as x_v^T[m,c] (lhsT = xf key-chunks); v_b enters
    via a rank-1 fp8 DoubleRow matmul (ones ⊗ v_b) into the same PSUM group.
  - softmax rowsum (sum over keys = partitions) is a ones-column bf16 matmul
    accumulated across the 8 key chunks into a [1,1024] PSUM; reciprocal ->
    bf16 row, broadcast to 128 partitions with gpsimd.partition_broadcast.
  - pass1 per key-chunk on DVE: attn = attT * rinv_bcast (all-bf16-SBUF
    tensor_tensor hits the 2x mode) + a tensor_scalar whose accumulator
    yields colsum; dinv and the x_v^T scale follow per chunk so the
    mc-streamed x_r matmuls unblock after chunk 0.
  - the key-side normalization 1/(1e-9+colsum) is folded into x_v^T as a
    per-partition scale (bf16 dynamic range absorbs the 1e9 amplification).
  - u = x_r - xq computed directly from PSUM (sign folded into negated wt).
  - BatchNorm (inference) folds into trans_w / trans_b on the host.

All matmuls run at 1 col/cycle (f32r with ap>=256, bf16); fp8 is avoided on
the value path (e4m3's 6% steps alone exceed the 2e-2 gate; so do fp8 q/k
projections and bf16 x via the exp's sensitivity to energy perturbations).
Real-HW constraints honoured: one PSUM bank per matmul output, no mixed
32/16-bit matmul inputs, f32r-marked producers, no gpsimd PSUM access, no
TensorTensor divide.

DMA: each descriptor costs ~625ns of serialized HWDGE issue and transfers
serialize at ~360GB/s, so weights take 4 descriptors total, x streams in
half-chunks interleaved with the projection/v-proj matmuls, and outputs
stream out per 512-col half as they finish.
"""

import numpy as np
from contextlib import ExitStack

import ml_dtypes
from concourse import bacc, tile, mybir
from concourse.bass_utils import run_bass_kernel_spmd

dt = mybir.dt
F32 = dt.float32
F32R = dt.float32r
BF16 = dt.bfloat16
F8 = dt.float8e4
AF = mybir.ActivationFunctionType
ALU = mybir.AluOpType
DR = mybir.MatmulPerfMode.DoubleRow

B = 16          # total batches
C = 512         # channels
NQ = 1024       # queries
NK = 1024       # keys
CQ = 128        # C // 4, q/k projection dim
NCORES = 8
BPC = B // NCORES  # batches per core

K_SHIFT = 20.0   # global energy shift replacing rowmax


def _r(ap):
    return ap.bitcast(F32R)


def build_program():
    nc = bacc.Bacc(
        "TRN2",
        target_bir_lowering=False,
        debug=False,
        enable_asserts=False,
        num_devices=NCORES,
    )

    x_d = nc.dram_tensor("x", [BPC, C, 2048], F32, kind="ExternalInput").ap()
    wq_d = nc.dram_tensor("wq", [128, 4, CQ], F32, kind="ExternalInput").ap()
    wv_d = nc.dram_tensor("wv", [128, 4, C], F32, kind="ExternalInput").ap()
    wt_d = nc.dram_tensor("wt", [128, 4, C], BF16, kind="ExternalInput").ap()
    sp_d = nc.dram_tensor("sp", [128, 2 * 8 + 4], F32, kind="ExternalInput").ap()
    vb8_d = nc.dram_tensor("vb8", [1, 2, C], F8, kind="ExternalInput").ap()
    out_d = nc.dram_tensor("out", [BPC, C, NQ], F32, kind="ExternalOutput").ap()

    with tile.TileContext(nc) as tc, ExitStack() as ctx:
        wp = ctx.enter_context(tc.tile_pool(name="w", bufs=1))
        xf_p = ctx.enter_context(tc.tile_pool(name="xf", bufs=2))
        proj_p = ctx.enter_context(tc.tile_pool(name="proj", bufs=2))
        att_p = ctx.enter_context(tc.tile_pool(name="att", bufs=3))
        xv_p = ctx.enter_context(tc.tile_pool(name="xv", bufs=3))
        u_p = ctx.enter_context(tc.tile_pool(name="u", bufs=1))
        rb_p = ctx.enter_context(tc.tile_pool(name="rb", bufs=2))
        st_p = ctx.enter_context(tc.tile_pool(name="st", bufs=2))
        out_p = ctx.enter_context(tc.tile_pool(name="outp", bufs=5))
        ps = ctx.enter_context(tc.tile_pool(name="ps", bufs=3, space="PSUM"))
        rs_p = ctx.enter_context(tc.tile_pool(name="rsp", bufs=1, space="PSUM"))

        # ---- replicated weights / constants ----
        wq = wp.tile([128, 4, CQ], F32)    # wq[p,j,d] = qk_w[d, j*128+p]
        wv = wp.tile([128, 4, C], F32)     # wv[p,j,c] = v_w[c, j*128+p]
        wt = wp.tile([128, 4, C], BF16)    # wt[p,j,c] = -(bn-folded trans_w)[c, j*128+p]
        sp = wp.tile([128, 20], F32)       # [kpb(b0) 8 | kpb(b1) 8 | tb2 4]
        vb8 = wp.tile([1, 2, C], F8)       # v_b row (fp8 DoubleRow rank-1)
        on1 = wp.tile([1, 2, 128], F8)     # DR ones/zeros pair
        onec = wp.tile([128, 1], BF16)     # ones column for rowsum matmul
        oner = wp.tile([1, 128], BF16)     # ones row for rinv broadcast matmul
        nc.vector.memset(on1[:, 0:1, :], 1.0)
        nc.vector.memset(on1[:, 1:2, :], 0.0)
        nc.vector.memset(onec, 1.0)
        nc.vector.memset(oner, 1.0)

        # input DMAs, half-chunk granular and ordered for earliest PE start
        nc.sync.dma_start(out=_r(wq), in_=_r(wq_d))
        nc.sync.dma_start(out=vb8, in_=vb8_d)
        S = {0: {}, 1: {}}
        for b in range(BPC):
            S[b]["xf"] = xf_p.tile([128, 4, 2048], F32, name=f"xf{b}", tag="xf")

        def xdma(b, j, hb):
            nc.sync.dma_start(
                out=_r(S[b]["xf"][:, j, hb * 1024:(hb + 1) * 1024]),
                in_=_r(x_d[b, j * 128:(j + 1) * 128,
                           hb * 1024:(hb + 1) * 1024]))

        xdma(0, 0, 0)
        xdma(0, 0, 1)
        nc.sync.dma_start(out=_r(wv), in_=_r(wv_d))
        for j in range(1, 4):
            for hb in range(2):
                xdma(0, j, hb)
        nc.sync.dma_start(out=sp, in_=sp_d)
        for j in range(4):
            for hb in range(2):
                xdma(1, j, hb)
        nc.sync.dma_start(out=wt, in_=wt_d)

        def vproj_pair(b, xv, pv, mcp):
            # finish one x_v^T mc-pair in a shared [128,1024] psum tile
            xf = S[b]["xf"]
            for r in range(2):
                mc = 2 * mcp + r
                for j in range(4):
                    nc.tensor.matmul(
                        out=pv[:, r * 512:(r + 1) * 512],
                        lhsT=_r(xf[:, j, 1024 + mc * 128:1024 + (mc + 1) * 128]),
                        rhs=_r(wv[:, j, :]),
                        start=(j == 0), stop=False,
                    )
                nc.tensor.matmul(  # += ones ⊗ v_b (fp8 DoubleRow rank-1)
                    out=pv[:, r * 512:(r + 1) * 512], lhsT=on1, rhs=vb8,
                    start=False, stop=True,
                    perf_mode=DR,
                )
            for r in range(2):
                mc = 2 * mcp + r
                nc.vector.tensor_copy(out=xv[:, mc, :],
                                      in_=pv[:, r * 512:(r + 1) * 512])

        def phase_a(b):
            s = S[b]
            xf = s["xf"]
            # q/k projections -> [d, n] / [d, m] (f32r), j-streamed with the
            # x DMA chunks; the first vproj mc-pair streams alongside.
            pq = ps.tile([128, 1024], F32, name=f"psq{b}", tag="ps")
            pk = ps.tile([128, 1024], F32, name=f"psk{b}", tag="ps")
            pv0 = ps.tile([128, 1024], F32, name=f"psv{b}_0", tag="ps")
            xv = s["xv"] = xv_p.tile([128, 8, C], BF16, name=f"xv{b}", tag="xv")
            for j in range(4):
                for h in range(2):
                    nc.tensor.matmul(
                        out=pq[:, h * 512:(h + 1) * 512],
                        lhsT=_r(wq[:, j, :]),
                        rhs=_r(xf[:, j, h * 512:(h + 1) * 512]),
                        start=(j == 0), stop=(j == 3),
                    )
                for h in range(2):
                    nc.tensor.matmul(
                        out=pk[:, h * 512:(h + 1) * 512],
                        lhsT=_r(wq[:, j, :]),
                        rhs=_r(xf[:, j, 1024 + h * 512:1024 + (h + 1) * 512]),
                        start=(j == 0), stop=(j == 3),
                    )
                for r in range(2):  # stream mc-pair 0 terms as chunks arrive
                    mc = r
                    nc.tensor.matmul(
                        out=pv0[:, r * 512:(r + 1) * 512],
                        lhsT=_r(xf[:, j, 1024 + mc * 128:1024 + (mc + 1) * 128]),
                        rhs=_r(wv[:, j, :]),
                        start=(j == 0), stop=False,
                    )
            for r in range(2):
                nc.tensor.matmul(
                    out=pv0[:, r * 512:(r + 1) * 512], lhsT=on1, rhs=vb8,
                    start=False, stop=True,
                    perf_mode=DR,
                )
            xqT = s["xqT"] = proj_p.tile([128, NQ], F32, name=f"xqT{b}", tag="xqT")
            xks = s["xks"] = proj_p.tile([128, NK], F32, name=f"xks{b}", tag="xks")
            nc.scalar.activation(out=_r(xqT), in_=pq, func=AF.Copy)
            nc.scalar.activation(out=_r(xks), in_=pk, func=AF.Copy)
            for r in range(2):
                nc.vector.tensor_copy(out=xv[:, r, :],
                                      in_=pv0[:, r * 512:(r + 1) * 512])
            for mcp in range(1, 4):
                pv = ps.tile([128, 1024], F32, name=f"psv{b}_{mcp}", tag="ps")
                vproj_pair(b, xv, pv, mcp)

        def phase_b(b):
            s = S[b]
            attT = s["attT"] = att_p.tile([128, 8, NK], BF16, name=f"attT{b}", tag="att")
            for mc in range(8):
                pe_t = ps.tile([128, 1024], F32, name=f"pse{b}_{mc}", tag="ps")
                for h in range(2):
                    nc.tensor.matmul(
                        out=pe_t[:, h * 512:(h + 1) * 512],
                        lhsT=_r(s["xks"][:, mc * 128:(mc + 1) * 128]),
                        rhs=_r(s["xqT"][:, h * 512:(h + 1) * 512]),
                        start=True, stop=True,
                    )
                # exp(E^T - kp[m] - K): per-partition bias, bf16 out
                nc.scalar.activation(
                    out=attT[:, mc, :], in_=pe_t, func=AF.Exp,
                    bias=sp[:, b * 8 + mc:b * 8 + mc + 1], scale=1.0,
                )
            # rowsum over keys: ones-column bf16 matmuls accumulated over mc
            rs = s["rs"] = rs_p.tile([1, 1024], F32, name=f"rs{b}", tag="rs")
            for mc in range(8):
                for h in range(2):
                    nc.tensor.matmul(
                        out=rs[:, h * 512:(h + 1) * 512],
                        lhsT=onec,
                        rhs=attT[:, mc, h * 512:(h + 1) * 512],
                        start=(mc == 0), stop=(mc == 7),
                    )

        def phase_c(b):
            s = S[b]
            # rowsum -> bf16 row (ACT), broadcast on gpsimd; pass1 is an
            # all-bf16 divide (DVE 2x mode) + in-place colsum accumulation
            rinv = rb_p.tile([1, 1024], BF16, name=f"ri{b}", tag="ri")
            with nc.allow_low_precision(reason="rinv at bf16: 0.4% rel, validated"):
                nc.vector.reciprocal(out=rinv, in_=s["rs"])
            rb = rb_p.tile([128, 1024], BF16, name=f"rb{b}", tag="rb")
            nc.gpsimd.partition_broadcast(out_ap=rb, in_ap=rinv, channels=128)
            attn = s["attn"] = att_p.tile([128, 8, NK], BF16, name=f"attn{b}", tag="att")
            cs = st_p.tile([128, 8], F32, name=f"cs{b}", tag="cs")
            dinv = st_p.tile([128, 8], F32, name=f"di{b}", tag="di")
            xvs = s["xvs"] = xv_p.tile([128, 8, C], BF16, name=f"xvs{b}", tag="xv")
            # per-chunk: pass1 (mult + colsum accum), then dinv and the
            # x_v^T scale immediately, so x_r's mc-streaming starts after
            # chunk 0 instead of after the whole pass
            for mc in range(8):
                nc.vector.tensor_tensor(out=attn[:, mc, :], in0=s["attT"][:, mc, :],
                                        in1=rb, op=ALU.mult)
                nc.vector.tensor_scalar(out=attn[:, mc, :], in0=attn[:, mc, :],
                                        scalar1=1.0, scalar2=0.0, op0=ALU.mult,
                                        op1=ALU.add, accum_out=cs[:, mc:mc + 1])
                nc.vector.tensor_scalar_add(out=cs[:, mc:mc + 1],
                                            in0=cs[:, mc:mc + 1], scalar1=1e-9)
                nc.vector.reciprocal(out=dinv[:, mc:mc + 1], in_=cs[:, mc:mc + 1])
                nc.vector.tensor_scalar_mul(out=xvs[:, mc, :], in0=s["xv"][:, mc, :],
                                            scalar1=dinv[:, mc:mc + 1])

        def phase_d(b):
            s = S[b]
            xf, xvs, attn = s["xf"], s["xvs"], s["attn"]
            u = u_p.tile([128, 4, NQ], BF16, name=f"u{b}", tag="u")
            for cp in range(2):
                # pair0 is mc-outer so pass1's streamed attn chunks unblock
                # it asap; pair1 runs later (attn complete) and goes cc-major
                # so its second psum tile isn't needed until the first u-sub
                # has long released a buffer
                prs = [ps.tile([128, 1024], F32, name=f"psr{b}_{cp}_{i}", tag="ps")
                       for i in range(2)]
                if cp == 0:
                    for mc in range(8):
                        for i, cc in enumerate((2 * cp, 2 * cp + 1)):
                            for h in range(2):
                                nc.tensor.matmul(
                                    out=prs[i][:, h * 512:(h + 1) * 512],
                                    lhsT=xvs[:, mc, cc * 128:(cc + 1) * 128],
                                    rhs=attn[:, mc, h * 512:(h + 1) * 512],
                                    start=(mc == 0), stop=(mc == 7),
                                )
                    for i, cc in enumerate((2 * cp, 2 * cp + 1)):
                        # u = x_r - xq (sign folded into negated wt)
                        nc.vector.tensor_sub(out=u[:, cc, :], in0=prs[i],
                                             in1=xf[:, cc, 0:1024])
                else:
                    for i, cc in enumerate((2 * cp, 2 * cp + 1)):
                        for mc in range(8):
                            for h in range(2):
                                nc.tensor.matmul(
                                    out=prs[i][:, h * 512:(h + 1) * 512],
                                    lhsT=xvs[:, mc, cc * 128:(cc + 1) * 128],
                                    rhs=attn[:, mc, h * 512:(h + 1) * 512],
                                    start=(mc == 0), stop=(mc == 7),
                                )
                        # u-sub right after this cc's group so it overlaps
                        # the next cc's matmuls
                        nc.vector.tensor_sub(out=u[:, cc, :], in0=prs[i],
                                             in1=xf[:, cc, 0:1024])

            nq_out = 2
            for cc in range(4):
                pt2 = ps.tile([128, 1024], F32, name=f"pso{b}_{cc}", tag="ps")
                ot = out_p.tile([128, NQ], F32, name=f"ot{b}_{cc}", tag="ot")
                for h in range(2):
                    for j in range(4):
                        nc.tensor.matmul(
                            out=pt2[:, h * 512:(h + 1) * 512],
                            lhsT=wt[:, j, cc * 128:(cc + 1) * 128],
                            rhs=u[:, j, h * 512:(h + 1) * 512],
                            start=(j == 0), stop=(j == 3),
                        )
                    w_q = 1024 // nq_out
                    for q in range(nq_out // 2):
                        qs = slice(h * 512 + q * w_q, h * 512 + (q + 1) * w_q)
                        nc.scalar.activation(out=ot[:, qs], in_=pt2[:, qs],
                                             func=AF.Relu,
                                             bias=sp[:, 16 + cc:16 + cc + 1],
                                             scale=1.0)
                        eng = nc.gpsimd if b == 0 else nc.vector
                        eng.tensor_add(out=ot[:, qs], in0=ot[:, qs],
                                       in1=xf[:, cc, qs])
                        nc.sync.dma_start(
                            out=out_d[b, cc * 128:(cc + 1) * 128, qs],
                            in_=ot[:, qs])

        phase_a(0)
        phase_b(0)
        phase_a(1)
        phase_b(1)
        phase_c(0)
        phase_d(0)
        phase_c(1)
        phase_d(1)

    nc.compile()
    return nc


def _host_prep(inputs):
    x = np.asarray(inputs["x"], np.float32)
    pos = np.asarray(inputs["pos"], np.float32)
    qk_w = np.asarray(inputs["qk_w"], np.float32)
    v_w = np.asarray(inputs["v_w"], np.float32)
    v_b = np.asarray(inputs["v_b"], np.float32)
    trans_w = np.asarray(inputs["trans_w"], np.float32)
    trans_b = np.asarray(inputs["trans_b"], np.float32)
    bn_gamma = np.asarray(inputs["bn_gamma"], np.float32)
    bn_beta = np.asarray(inputs["bn_beta"], np.float32)
    bn_mean = np.asarray(inputs["bn_mean"], np.float32)
    bn_var = np.asarray(inputs["bn_var"], np.float32)
    pos_w = np.asarray(inputs["pos_w"], np.float32)

    a = bn_gamma / np.sqrt(bn_var + 1e-5)
    wt2 = a[:, None] * trans_w
    tb2 = a * trans_b + bn_beta - a * bn_mean
    # per-key positional projection; q_proj/pos_b cancel in the key softmax
    kp = np.einsum("bpm,p->bm", pos[:, :, NQ:], pos_w).astype(np.float32)

    wq_h = np.ascontiguousarray(
        qk_w.T.reshape(4, 128, CQ).transpose(1, 0, 2))
    wv_h = np.ascontiguousarray(
        v_w.T.reshape(4, 128, C).transpose(1, 0, 2))
    wt_h = np.ascontiguousarray(
        (-wt2.T).reshape(4, 128, C).transpose(1, 0, 2)).astype(ml_dtypes.bfloat16)
    vb8 = np.zeros((1, 2, C), ml_dtypes.float8_e4m3)
    vb8[0, 0, :] = v_b.astype(ml_dtypes.float8_e4m3)

    common = {"wq": wq_h, "wv": wv_h, "wt": wt_h, "vb8": vb8}
    in_maps = []
    for i in range(NCORES):
        m = dict(common)
        m["x"] = np.ascontiguousarray(x[BPC * i:BPC * (i + 1)])
        sp_h = np.zeros((128, 20), np.float32)
        for b in range(BPC):
            kb = kp[BPC * i + b]  # [NK]
            sp_h[:, b * 8:(b + 1) * 8] = -(kb.reshape(8, 128).T + K_SHIFT)
        sp_h[:, 16:20] = tb2.reshape(4, 128).T
        m["sp"] = sp_h
        in_maps.append(m)
    return in_maps


_PROGRAM = None


def kernel(**inputs):
    global _PROGRAM
    in_maps = _host_prep(inputs)
    if _PROGRAM is None:
        _PROGRAM = build_program()
    res = run_bass_kernel_spmd(_PROGRAM, in_maps, list(range(NCORES)))
    out = np.concatenate([r["out"] for r in res.results], axis=0)
    return np.ascontiguousarray(out, dtype=np.float32)


# revision 62
# speedup vs baseline: 1.0098x; 1.0098x over previous
"""Coarse-Fine self-attention layer on 8 Trainium2 NeuronCores.

Data-parallel over batch: 16 batches -> 2 per core. Weights replicated.

Transposed-attention formulation (keys on partitions everywhere):
  - energy is computed TRANSPOSED: E^T[m,n] = x_k^T x_q via lhsT=xks chunks,
    so the per-key positional term kp[m] and the global K shift ride the
    activation-exp bias (per-partition) for free, and no SBUF<->SBUF DMA
    transposes of att/x_v are needed at all.
  - x_v is produced directly as x_v^T[m,c] (lhsT = xf key-chunks); v_b enters
    via a rank-1 fp8 DoubleRow matmul (ones ⊗ v_b) into the same PSUM group.
  - softmax rowsum (sum over keys = partitions) is a ones-column bf16 matmul
    accumulated across the 8 key chunks into a [1,1024] PSUM; reciprocal ->
    bf16 row, broadcast to 128 partitions with gpsimd.partition_broadcast.
  - pass1 per key-chunk on DVE: attn = attT * rinv_bcast (all-bf16-SBUF
    tensor_tensor hits the 2x mode) + a tensor_scalar whose accumulator
    yields colsum; dinv and the x_v^T scale follow per chunk so the
    mc-streamed x_r matmuls unblock after chunk 0.
  - the key-side normalization 1/(1e-9+colsum) is folded into x_v^T as a
    per-partition scale (bf16 dynamic range absorbs the 1e9 amplification).
  - u = x_r - xq computed directly from PSUM (sign folded into negated wt).
  - BatchNorm (inference) folds into trans_w / trans_b on the host.

All matmuls run at 1 col/cycle (f32r with ap>=256, bf16); fp8 is avoided on
the value path (e4m3's 6% steps alone exceed the 2e-2 gate; so do fp8 q/k
projections and bf16 x via the exp's sensitivity to energy perturbations).
Real-HW constraints honoured: one PSUM bank per matmul output, no mixed
32/16-bit matmul inputs, f32r-marked producers, no gpsimd PSUM access, no
TensorTensor divide.

DMA: each descriptor costs ~625ns of serialized HWDGE issue and transfers
serialize at ~360GB/s, so weights take 4 descriptors total, x streams in
half-chunks interleaved with the projection/v-proj matmuls, and outputs
stream out per 512-col half as they finish.
"""

import numpy as np
from contextlib import ExitStack

import ml_dtypes
from concourse import bacc, tile, mybir
from concourse.bass_utils import run_bass_kernel_spmd

dt = mybir.dt
F32 = dt.float32
F32R = dt.float32r
BF16 = dt.bfloat16
F8 = dt.float8e4
AF = mybir.ActivationFunctionType
ALU = mybir.AluOpType
DR = mybir.MatmulPerfMode.DoubleRow

B = 16          # total batches
C = 512         # channels
NQ = 1024       # queries
NK = 1024       # keys
CQ = 128        # C // 4, q/k projection dim
NCORES = 8
BPC = B // NCORES  # batches per core

K_SHIFT = 20.0   # global energy shift replacing rowmax


def _r(ap):
    return ap.bitcast(F32R)


def build_program():
    nc = bacc.Bacc(
        "TRN2",
        target_bir_lowering=False,
        debug=False,
        enable_asserts=False,
        num_devices=NCORES,
    )

    x_d = nc.dram_tensor("x", [BPC, C, 2048], F32, kind="ExternalInput").ap()
    wq_d = nc.dram_tensor("wq", [128, 4, CQ], F32, kind="ExternalInput").ap()
    wv_d = nc.dram_tensor("wv", [128, 4, C], F32, kind="ExternalInput").ap()
    wt_d = nc.dram_tensor("wt", [128, 4, C], BF16, kind="ExternalInput").ap()
    sp_d = nc.dram_tensor("sp", [128, 2 * 8 + 4], F32, kind="ExternalInput").ap()
    vb8_d = nc.dram_tensor("vb8", [1, 2, C], F8, kind="ExternalInput").ap()
    out_d = nc.dram_tensor("out", [BPC, C, NQ], F32, kind="ExternalOutput").ap()

    with tile.TileContext(nc) as tc, ExitStack() as ctx:
        wp = ctx.enter_context(tc.tile_pool(name="w", bufs=1))
        xf_p = ctx.enter_context(tc.tile_pool(name="xf", bufs=2))
        proj_p = ctx.enter_context(tc.tile_pool(name="proj", bufs=2))
        att_p = ctx.enter_context(tc.tile_pool(name="att", bufs=3))
        xv_p = ctx.enter_context(tc.tile_pool(name="xv", bufs=3))
        u_p = ctx.enter_context(tc.tile_pool(name="u", bufs=1))
        rb_p = ctx.enter_context(tc.tile_pool(name="rb", bufs=2))
        st_p = ctx.enter_context(tc.tile_pool(name="st", bufs=2))
        out_p = ctx.enter_context(tc.tile_pool(name="outp", bufs=5))
        ps = ctx.enter_context(tc.tile_pool(name="ps", bufs=3, space="PSUM"))
        rs_p = ctx.enter_context(tc.tile_pool(name="rsp", bufs=1, space="PSUM"))

        # ---- replicated weights / constants ----
        wq = wp.tile([128, 4, CQ], F32)    # wq[p,j,d] = qk_w[d, j*128+p]
        wv = wp.tile([128, 4, C], F32)     # wv[p,j,c] = v_w[c, j*128+p]
        wt = wp.tile([128, 4, C], BF16)    # wt[p,j,c] = -(bn-folded trans_w)[c, j*128+p]
        sp = wp.tile([128, 20], F32)       # [kpb(b0) 8 | kpb(b1) 8 | tb2 4]
        vb8 = wp.tile([1, 2, C], F8)       # v_b row (fp8 DoubleRow rank-1)
        on1 = wp.tile([1, 2, 128], F8)     # DR ones/zeros pair
        onec = wp.tile([128, 1], BF16)     # ones column for rowsum matmul
        oner = wp.tile([1, 128], BF16)     # ones row for rinv broadcast matmul
        nc.vector.memset(on1[:, 0:1, :], 1.0)
        nc.vector.memset(on1[:, 1:2, :], 0.0)
        nc.vector.memset(onec, 1.0)
        nc.vector.memset(oner, 1.0)

        # input DMAs, half-chunk granular and ordered for earliest PE start
        nc.sync.dma_start(out=_r(wq), in_=_r(wq_d))
        nc.sync.dma_start(out=vb8, in_=vb8_d)
        S = {0: {}, 1: {}}
        for b in range(BPC):
            S[b]["xf"] = xf_p.tile([128, 4, 2048], F32, name=f"xf{b}", tag="xf")

        def xdma(b, j, hb):
            nc.sync.dma_start(
                out=_r(S[b]["xf"][:, j, hb * 1024:(hb + 1) * 1024]),
                in_=_r(x_d[b, j * 128:(j + 1) * 128,
                           hb * 1024:(hb + 1) * 1024]))

        xdma(0, 0, 0)
        xdma(0, 0, 1)
        nc.sync.dma_start(out=_r(wv), in_=_r(wv_d))
        for j in range(1, 4):
            for hb in range(2):
                xdma(0, j, hb)
        nc.sync.dma_start(out=sp, in_=sp_d)
        for j in range(4):
            for hb in range(2):
                xdma(1, j, hb)
        nc.sync.dma_start(out=wt, in_=wt_d)

        def vproj_pair(b, xv, pv, mcp):
            # finish one x_v^T mc-pair in a shared [128,1024] psum tile
            xf = S[b]["xf"]
            for r in range(2):
                mc = 2 * mcp + r
                for j in range(4):
                    nc.tensor.matmul(
                        out=pv[:, r * 512:(r + 1) * 512],
                        lhsT=_r(xf[:, j, 1024 + mc * 128:1024 + (mc + 1) * 128]),
                        rhs=_r(wv[:, j, :]),
                        start=(j == 0), stop=False,
                    )
                nc.tensor.matmul(  # += ones ⊗ v_b (fp8 DoubleRow rank-1)
                    out=pv[:, r * 512:(r + 1) * 512], lhsT=on1, rhs=vb8,
                    start=False, stop=True,
                    perf_mode=DR,
                )
            for r in range(2):
                mc = 2 * mcp + r
                nc.vector.tensor_copy(out=xv[:, mc, :],
                                      in_=pv[:, r * 512:(r + 1) * 512])

        def phase_a(b):
            s = S[b]
            xf = s["xf"]
            # q/k projections -> [d, n] / [d, m] (f32r), j-streamed with the
            # x DMA chunks; the first vproj mc-pair streams alongside.
            pq = ps.tile([128, 1024], F32, name=f"psq{b}", tag="ps")
            pk = ps.tile([128, 1024], F32, name=f"psk{b}", tag="ps")
            pv0 = ps.tile([128, 1024], F32, name=f"psv{b}_0", tag="ps")
            xv = s["xv"] = xv_p.tile([128, 8, C], BF16, name=f"xv{b}", tag="xv")
            for j in range(4):
                for h in range(2):
                    nc.tensor.matmul(
                        out=pq[:, h * 512:(h + 1) * 512],
                        lhsT=_r(wq[:, j, :]),
                        rhs=_r(xf[:, j, h * 512:(h + 1) * 512]),
                        start=(j == 0), stop=(j == 3),
                    )
                for h in range(2):
                    nc.tensor.matmul(
                        out=pk[:, h * 512:(h + 1) * 512],
                        lhsT=_r(wq[:, j, :]),
                        rhs=_r(xf[:, j, 1024 + h * 512:1024 + (h + 1) * 512]),
                        start=(j == 0), stop=(j == 3),
                    )
                for r in range(2):  # stream mc-pair 0 terms as chunks arrive
                    mc = r
                    nc.tensor.matmul(
                        out=pv0[:, r * 512:(r + 1) * 512],
                        lhsT=_r(xf[:, j, 1024 + mc * 128:1024 + (mc + 1) * 128]),
                        rhs=_r(wv[:, j, :]),
                        start=(j == 0), stop=False,
                    )
            for r in range(2):
                nc.tensor.matmul(
                    out=pv0[:, r * 512:(r + 1) * 512], lhsT=on1, rhs=vb8,
                    start=False, stop=True,
                    perf_mode=DR,
                )
            xqT = s["xqT"] = proj_p.tile([128, NQ], F32, name=f"xqT{b}", tag="xqT")
            xks = s["xks"] = proj_p.tile([128, NK], F32, name=f"xks{b}", tag="xks")
            nc.scalar.activation(out=_r(xqT), in_=pq, func=AF.Copy)
            nc.scalar.activation(out=_r(xks), in_=pk, func=AF.Copy)
            for r in range(2):
                nc.vector.tensor_copy(out=xv[:, r, :],
                                      in_=pv0[:, r * 512:(r + 1) * 512])
            for mcp in range(1, 4):
                pv = ps.tile([128, 1024], F32, name=f"psv{b}_{mcp}", tag="ps")
                vproj_pair(b, xv, pv, mcp)

        def phase_b(b, rowsum=True):
            s = S[b]
            attT = s["attT"] = att_p.tile([128, 8, NK], BF16, name=f"attT{b}", tag="att")
            for mc in range(8):
                pe_t = ps.tile([128, 1024], F32, name=f"pse{b}_{mc}", tag="ps")
                for h in range(2):
                    nc.tensor.matmul(
                        out=pe_t[:, h * 512:(h + 1) * 512],
                        lhsT=_r(s["xks"][:, mc * 128:(mc + 1) * 128]),
                        rhs=_r(s["xqT"][:, h * 512:(h + 1) * 512]),
                        start=True, stop=True,
                    )
                # exp(E^T - kp[m] - K): per-partition bias, bf16 out
                nc.scalar.activation(
                    out=attT[:, mc, :], in_=pe_t, func=AF.Exp,
                    bias=sp[:, b * 8 + mc:b * 8 + mc + 1], scale=1.0,
                )
            if not rowsum:
                return
            phase_rowsum(b)

        def phase_rowsum(b):
            s = S[b]
            attT = s["attT"]
            # rowsum over keys: ones-column bf16 matmuls accumulated over mc
            rs = s["rs"] = rs_p.tile([1, 1024], F32, name=f"rs{b}", tag="rs")
            for mc in range(8):
                for h in range(2):
                    nc.tensor.matmul(
                        out=rs[:, h * 512:(h + 1) * 512],
                        lhsT=onec,
                        rhs=attT[:, mc, h * 512:(h + 1) * 512],
                        start=(mc == 0), stop=(mc == 7),
                    )

        def phase_c(b):
            s = S[b]
            # rowsum -> bf16 row (ACT), broadcast on gpsimd; pass1 is an
            # all-bf16 divide (DVE 2x mode) + in-place colsum accumulation
            rinv = rb_p.tile([1, 1024], BF16, name=f"ri{b}", tag="ri")
            with nc.allow_low_precision(reason="rinv at bf16: 0.4% rel, validated"):
                nc.vector.reciprocal(out=rinv, in_=s["rs"])
            rb = rb_p.tile([128, 1024], BF16, name=f"rb{b}", tag="rb")
            nc.gpsimd.partition_broadcast(out_ap=rb, in_ap=rinv, channels=128)
            attn = s["attn"] = att_p.tile([128, 8, NK], BF16, name=f"attn{b}", tag="att")
            cs = st_p.tile([128, 8], F32, name=f"cs{b}", tag="cs")
            dinv = st_p.tile([128, 8], F32, name=f"di{b}", tag="di")
            xvs = s["xvs"] = xv_p.tile([128, 8, C], BF16, name=f"xvs{b}", tag="xv")
            # per-chunk: pass1 (mult + colsum accum), then dinv and the
            # x_v^T scale immediately, so x_r's mc-streaming starts after
            # chunk 0 instead of after the whole pass
            for mc in range(8):
                nc.vector.tensor_tensor(out=attn[:, mc, :], in0=s["attT"][:, mc, :],
                                        in1=rb, op=ALU.mult)
                nc.vector.tensor_scalar(out=attn[:, mc, :], in0=attn[:, mc, :],
                                        scalar1=1.0, scalar2=0.0, op0=ALU.mult,
                                        op1=ALU.add, accum_out=cs[:, mc:mc + 1])
                nc.vector.tensor_scalar_add(out=cs[:, mc:mc + 1],
                                            in0=cs[:, mc:mc + 1], scalar1=1e-9)
                nc.vector.reciprocal(out=dinv[:, mc:mc + 1], in_=cs[:, mc:mc + 1])
                nc.vector.tensor_scalar_mul(out=xvs[:, mc, :], in0=s["xv"][:, mc, :],
                                            scalar1=dinv[:, mc:mc + 1])

        def phase_d(b, xr=True, trans=True):
            s = S[b]
            xf, xvs, attn = s["xf"], s["xvs"], s["attn"]
            if xr:
                s["u"] = u_p.tile([128, 4, NQ], BF16, name=f"u{b}", tag="u")
            u = s["u"]
            if not xr:
                trans_only(b, u, xf)
                return
            for cp in range(2):
                # pair0 is mc-outer so pass1's streamed attn chunks unblock
                # it asap; pair1 runs later (attn complete) and goes cc-major
                # so its second psum tile isn't needed until the first u-sub
                # has long released a buffer
                prs = [ps.tile([128, 1024], F32, name=f"psr{b}_{cp}_{i}", tag="ps")
                       for i in range(2)]
                if cp == 0:
                    for mc in range(8):
                        for i, cc in enumerate((2 * cp, 2 * cp + 1)):
                            for h in range(2):
                                nc.tensor.matmul(
                                    out=prs[i][:, h * 512:(h + 1) * 512],
                                    lhsT=xvs[:, mc, cc * 128:(cc + 1) * 128],
                                    rhs=attn[:, mc, h * 512:(h + 1) * 512],
                                    start=(mc == 0), stop=(mc == 7),
                                )
                    for i, cc in enumerate((2 * cp, 2 * cp + 1)):
                        # u = x_r - xq (sign folded into negated wt)
                        nc.vector.tensor_sub(out=u[:, cc, :], in0=prs[i],
                                             in1=xf[:, cc, 0:1024])
                else:
                    for i, cc in enumerate((2 * cp, 2 * cp + 1)):
                        for mc in range(8):
                            for h in range(2):
                                nc.tensor.matmul(
                                    out=prs[i][:, h * 512:(h + 1) * 512],
                                    lhsT=xvs[:, mc, cc * 128:(cc + 1) * 128],
                                    rhs=attn[:, mc, h * 512:(h + 1) * 512],
                                    start=(mc == 0), stop=(mc == 7),
                                )
                        # u-sub right after this cc's group so it overlaps
                        # the next cc's matmuls
                        nc.vector.tensor_sub(out=u[:, cc, :], in0=prs[i],
                                             in1=xf[:, cc, 0:1024])

            if not trans:
                return
            trans_only(b, u, xf)

        def trans_only(b, u, xf):
            nq_out = 2
            for cc in range(4):
                pt2 = ps.tile([128, 1024], F32, name=f"pso{b}_{cc}", tag="ps")
                ot = out_p.tile([128, NQ], F32, name=f"ot{b}_{cc}", tag="ot")
                for h in range(2):
                    for j in range(4):
                        nc.tensor.matmul(
                            out=pt2[:, h * 512:(h + 1) * 512],
                            lhsT=wt[:, j, cc * 128:(cc + 1) * 128],
                            rhs=u[:, j, h * 512:(h + 1) * 512],
                            start=(j == 0), stop=(j == 3),
                        )
                    w_q = 1024 // nq_out
                    for q in range(nq_out // 2):
                        qs = slice(h * 512 + q * w_q, h * 512 + (q + 1) * w_q)
                        nc.scalar.activation(out=ot[:, qs], in_=pt2[:, qs],
                                             func=AF.Relu,
                                             bias=sp[:, 16 + cc:16 + cc + 1],
                                             scale=1.0)
                        eng = nc.gpsimd if b == 0 else nc.vector
                        eng.tensor_add(out=ot[:, qs], in0=ot[:, qs],
                                       in1=xf[:, cc, qs])
                        nc.sync.dma_start(
                            out=out_d[b, cc * 128:(cc + 1) * 128, qs],
                            in_=ot[:, qs])

        phase_a(0)
        phase_b(0)
        phase_a(1)
        phase_b(1, rowsum=False)
        phase_c(0)
        phase_d(0, trans=False)
        phase_rowsum(1)
        phase_d(0, xr=False)
        phase_c(1)
        phase_d(1)

    nc.compile()
    return nc


def _host_prep(inputs):
    x = np.asarray(inputs["x"], np.float32)
    pos = np.asarray(inputs["pos"], np.float32)
    qk_w = np.asarray(inputs["qk_w"], np.float32)
    v_w = np.asarray(inputs["v_w"], np.float32)
    v_b = np.asarray(inputs["v_b"], np.float32)
    trans_w = np.asarray(inputs["trans_w"], np.float32)
    trans_b = np.asarray(inputs["trans_b"], np.float32)
    bn_gamma = np.asarray(inputs["bn_gamma"], np.float32)
    bn_beta = np.asarray(inputs["bn_beta"], np.float32)
    bn_mean = np.asarray(inputs["bn_mean"], np.float32)
    bn_var = np.asarray(inputs["bn_var"], np.float32)
    pos_w = np.asarray(inputs["pos_w"], np.float32)

    a = bn_gamma / np.sqrt(bn_var + 1e-5)
    wt2 = a[:, None] * trans_w
    tb2 = a * trans_b + bn_beta - a * bn_mean
    # per-key positional projection; q_proj/pos_b cancel in the key softmax
    kp = np.einsum("bpm,p->bm", pos[:, :, NQ:], pos_w).astype(np.float32)

    wq_h = np.ascontiguousarray(
        qk_w.T.reshape(4, 128, CQ).transpose(1, 0, 2))
    wv_h = np.ascontiguousarray(
        v_w.T.reshape(4, 128, C).transpose(1, 0, 2))
    wt_h = np.ascontiguousarray(
        (-wt2.T).reshape(4, 128, C).transpose(1, 0, 2)).astype(ml_dtypes.bfloat16)
    vb8 = np.zeros((1, 2, C), ml_dtypes.float8_e4m3)
    vb8[0, 0, :] = v_b.astype(ml_dtypes.float8_e4m3)

    common = {"wq": wq_h, "wv": wv_h, "wt": wt_h, "vb8": vb8}
    in_maps = []
    for i in range(NCORES):
        m = dict(common)
        m["x"] = np.ascontiguousarray(x[BPC * i:BPC * (i + 1)])
        sp_h = np.zeros((128, 20), np.float32)
        for b in range(BPC):
            kb = kp[BPC * i + b]  # [NK]
            sp_h[:, b * 8:(b + 1) * 8] = -(kb.reshape(8, 128).T + K_SHIFT)
        sp_h[:, 16:20] = tb2.reshape(4, 128).T
        m["sp"] = sp_h
        in_maps.append(m)
    return in_maps


_PROGRAM = None


def kernel(**inputs):
    global _PROGRAM
    in_maps = _host_prep(inputs)
    if _PROGRAM is None:
        _PROGRAM = build_program()
    res = run_bass_kernel_spmd(_PROGRAM, in_maps, list(range(NCORES)))
    out = np.concatenate([r["out"] for r in res.results], axis=0)
    return np.ascontiguousarray(out, dtype=np.float32)
